# revision 13
# baseline (speedup 1.0000x reference)
"""Trainium2 Bass kernel for nn_Ir_Consistency_Loss (gnn_message_passing).

loss = mean_e (1 - re[src_e].re[dst_e]) * ||ir_h[src_e] - ir_h[dst_e]||^2

Edge-parallel across 8 NeuronCores, dma_gather-based:
  - Host: gather table G = concat(re_, ir_h) [N, 256] cast to bf16, split
    into halves GA = G[:25000], GB = G[25000:] so local row ids fit
    dma_gather's int16. A zero row is appended to each half; pad edges
    index it on both endpoints, so their loss contribution is exactly 0
    (no host-side correction needed).
  - Edges bucketed by (src-half, dst-half) into 4 streams; each bucket is
    sharded over 8 cores and padded to a common per-core tile count.
  - Device, per tile of 4096 edges: two dma_gathers in PREPARE_ONLY mode
    (desc-gen on the Pool/Q7 engine) + trigger_dma, so descriptor
    generation for tile t+1 overlaps the DMA transfer of tile t.
    Rows are 512B (256 x bf16).
  - DVE (bf16 inputs): prod = u_re*v_re -> reduce agree (f32);
    diff = u_ir - v_ir; ACT engine squares diff; reduce sqsum (f32);
    scalar_tensor_tensor computes (agree-1)*sqsum accumulated into
    per-tile partials [negated tile loss].
  - Host: loss = -(sum of partials) / E.

dma_gather applies the fixed bijection j -> out[j%128, j//128] with index
SBUF layout j -> [j%16, j//16] replicated on the 8 16-partition groups
(HW-verified). src and dst use the identical layout, so per-edge slots align
and the final sum is permutation-invariant.
"""

import numpy as np
import ml_dtypes

import concourse.bacc as bacc
import concourse.mybir as mybir
import concourse.tile as tile
from concourse.bass_utils import run_bass_kernel_spmd

N_NODES = 50000
HALF = 25000
D = 128
N_CORES = 8
P = 128
SLOTS = 32                 # edges per partition per tile
TILE_E = P * SLOTS         # 4096 edges per tile
IDX_COLS = TILE_E // 16    # int16 idx columns (wrap-16 layout)
PAD_ROW = HALF             # local id of the all-zero pad row in each table
TBL_ROWS = HALF + 8        # table rows padded for alignment

USE_PREP = True            # prepare_only + trigger_dma pipelining
IDX_BUFS = 3               # idx pool depth (slot reuse distance)
GATH_BUFS = 3              # gather pool depth

_cache = {}


def _build_program(tiles_per_bucket):
    key = tuple(tiles_per_bucket)
    if key in _cache:
        return _cache[key]
    total_tiles = sum(tiles_per_bucket)
    nc = bacc.Bacc("TRN2", target_bir_lowering=False, debug=False,
                   num_devices=N_CORES)
    bf16 = mybir.dt.bfloat16
    fp32 = mybir.dt.float32
    ga = nc.dram_tensor("ga", [TBL_ROWS, 2 * D], bf16, kind="ExternalInput")
    gb = nc.dram_tensor("gb", [TBL_ROWS, 2 * D], bf16, kind="ExternalInput")
    eidx = nc.dram_tensor("edge_idx", [total_tiles * P, 2 * IDX_COLS],
                          mybir.dt.int16, kind="ExternalInput")
    out = nc.dram_tensor("partial", [P, 1], mybir.dt.float32,
                         kind="ExternalOutput")

    Alu = mybir.AluOpType
    X = mybir.AxisListType.X
    Sq = mybir.ActivationFunctionType.Square
    stab = [ga, ga, gb, gb]
    dtab = [ga, gb, ga, gb]

    with tile.TileContext(nc) as tc:
        # One DMA-completion semaphore per tile: target is exactly 32 (2
        # preps x 16 engines), so the consumer wait is airtight regardless
        # of cross-engine skew, unlike cumulative targets on a shared sem.
        gsems = ([nc.alloc_semaphore(f"g{i}") for i in range(total_tiles)]
                 if USE_PREP else None)
        with (
            tc.tile_pool(name="idx", bufs=IDX_BUFS) as ipool,
            tc.tile_pool(name="gath", bufs=GATH_BUFS) as gpool,
            tc.tile_pool(name="scr", bufs=2) as spool,
            tc.tile_pool(name="stats", bufs=1) as stpool,
        ):
            partials = stpool.tile([P, total_tiles], fp32, tag="partials")
            t = 0
            for b in range(4):
                for _ in range(tiles_per_bucket[b]):
                    ei = ipool.tile([P, 2 * IDX_COLS], mybir.dt.int16,
                                    tag="ei")
                    if USE_PREP and t >= IDX_BUFS:
                        # WAR vs the prep that read the idx slot being
                        # reused: its (deferred) side effects are attributed
                        # to its DMA completion sem.
                        nc.gpsimd.wait_ge(gsems[t - IDX_BUFS], 32)
                    nc.gpsimd.dma_start(out=ei[:],
                                        in_=eidx[t * P:(t + 1) * P, :])
                    si = ei[:, 0:IDX_COLS]
                    di = ei[:, IDX_COLS:2 * IDX_COLS]

                    u = gpool.tile([P, SLOTS, 2 * D], bf16, tag="u")
                    v = gpool.tile([P, SLOTS, 2 * D], bf16, tag="v")
                    if USE_PREP:
                        nc.gpsimd.dma_gather(u[:], stab[b][:], si, TILE_E,
                                             TILE_E, 2 * D, single_packet=False,
                                             prepare_only=True, sem=gsems[t])
                        nc.gpsimd.dma_gather(v[:], dtab[b][:], di, TILE_E,
                                             TILE_E, 2 * D, single_packet=False,
                                             prepare_only=True, sem=gsems[t])
                        nc.gpsimd.trigger_dma(count=None)
                    else:
                        nc.gpsimd.dma_gather(u[:], stab[b][:], si, TILE_E,
                                             TILE_E, 2 * D, single_packet=False)
                        nc.gpsimd.dma_gather(v[:], dtab[b][:], di, TILE_E,
                                             TILE_E, 2 * D, single_packet=False)

                    prod = spool.tile([P, SLOTS, D], bf16, tag="prod")
                    agree = spool.tile([P, SLOTS], fp32, tag="agree")
                    diff = spool.tile([P, SLOTS, D], bf16, tag="diff")
                    sq = spool.tile([P, SLOTS, D], bf16, tag="sq")
                    sqsum = spool.tile([P, SLOTS], fp32, tag="sqsum")
                    junk = spool.tile([P, SLOTS], fp32, tag="junk")

                    if USE_PREP:
                        # manual RAW sync: Tile does not gate consumers on a
                        # prepared gather's DMA completion; each prep's
                        # descriptors bump the tile sem by 16 (one/engine).
                        nc.vector.wait_ge(gsems[t], 32)
                    nc.vector.tensor_tensor(out=prod[:], in0=u[:, :, 0:D],
                                            in1=v[:, :, 0:D], op=Alu.mult)
                    nc.vector.tensor_reduce(out=agree[:], in_=prod[:], axis=X,
                                            op=Alu.add)
                    nc.vector.tensor_tensor(out=diff[:], in0=u[:, :, D:2 * D],
                                            in1=v[:, :, D:2 * D],
                                            op=Alu.subtract)
                    nc.scalar.activation(out=sq[:], in_=diff[:], func=Sq)
                    nc.vector.tensor_reduce(out=sqsum[:], in_=sq[:], axis=X,
                                            op=Alu.add)
                    nc.vector.scalar_tensor_tensor(
                        out=junk[:], in0=agree[:], scalar=1.0, in1=sqsum[:],
                        op0=Alu.subtract, op1=Alu.mult,
                        accum_out=partials[:, t:t + 1])
                    t += 1

            total = stpool.tile([P, 1], fp32, tag="total")
            nc.vector.tensor_reduce(out=total[:], in_=partials[:], axis=X,
                                    op=Alu.add)
            nc.sync.dma_start(out=out[:], in_=total[:])
    nc.compile()
    _cache[key] = nc
    return nc


def _wrap_idx(flat_idx):
    """[n_tiles, TILE_E] local ids -> [n_tiles*P, IDX_COLS] int16 blocks.
    Logical j -> [j % 16, j // 16], replicated on all 8 16-row groups."""
    nt = flat_idx.shape[0]
    j = np.arange(TILE_E)
    w = np.zeros((nt, 16, IDX_COLS), np.int16)
    w[:, j % 16, j // 16] = flat_idx.astype(np.int16)
    return np.ascontiguousarray(np.tile(w, (1, 8, 1))).reshape(nt * P, IDX_COLS)


def kernel(re_, ir_h, src, dst):
    re_ = np.asarray(re_, dtype=np.float32)
    ir_h = np.asarray(ir_h, dtype=np.float32)
    g = np.concatenate([re_, ir_h], axis=1).astype(ml_dtypes.bfloat16)
    ga = np.zeros((TBL_ROWS, 2 * D), ml_dtypes.bfloat16)
    gb = np.zeros((TBL_ROWS, 2 * D), ml_dtypes.bfloat16)
    ga[:HALF] = g[:HALF]
    gb[:HALF] = g[HALF:]

    s = np.asarray(src).astype(np.int64)
    d = np.asarray(dst).astype(np.int64)
    e_total = s.shape[0]
    bucket = (s >= HALF) * 2 + (d >= HALF)

    src_blocks = [[] for _ in range(N_CORES)]
    dst_blocks = [[] for _ in range(N_CORES)]
    tiles_per_bucket = []
    for b in range(4):
        m = bucket == b
        sb = s[m] - (HALF if b >= 2 else 0)
        db = d[m] - (HALF if b % 2 == 1 else 0)
        n = sb.shape[0]
        per_core = -(-n // N_CORES)
        nt = max(1, -(-per_core // TILE_E))
        tiles_per_bucket.append(nt)
        cap = nt * TILE_E
        sp = np.full(cap * N_CORES, PAD_ROW, np.int64)
        dp = np.full(cap * N_CORES, PAD_ROW, np.int64)
        sp[:n] = sb
        dp[:n] = db
        for c in range(N_CORES):
            src_blocks[c].append(sp[c * cap:(c + 1) * cap].reshape(nt, TILE_E))
            dst_blocks[c].append(dp[c * cap:(c + 1) * cap].reshape(nt, TILE_E))

    in_maps = []
    nt_total = sum(tiles_per_bucket)
    for c in range(N_CORES):
        sw = _wrap_idx(np.concatenate(src_blocks[c], axis=0))
        dw = _wrap_idx(np.concatenate(dst_blocks[c], axis=0))
        ei = np.concatenate([sw.reshape(nt_total, P, IDX_COLS),
                             dw.reshape(nt_total, P, IDX_COLS)], axis=2)
        ei = np.ascontiguousarray(ei).reshape(nt_total * P, 2 * IDX_COLS)
        in_maps.append({"ga": ga, "gb": gb, "edge_idx": ei})

    nc = _build_program(tuple(tiles_per_bucket))
    res = run_bass_kernel_spmd(nc, in_maps, core_ids=list(range(N_CORES)))
    tot = 0.0
    for r in res.results:
        tot += float(r["partial"].sum(dtype=np.float64))
    return np.float32(-tot / e_total)
